# revision 1
# baseline (speedup 1.0000x reference)
"""Two-layer GAT (GATConv(128->4x64, concat) + LayerNorm + ELU +
GATConv(256->2)) on 8 trn2 NeuronCores via Bass/Tile.

Distribution (graph/data parallel per the sharding hint): destination nodes
are partitioned across 8 cores; weights are replicated; each core computes the
dense node transform for all nodes (cheap) so no halo exchange is needed for
layer 0; layer-1 activations are AllGathered once at the layer boundary.

Core algorithm per layer, per core:
- Edges (incl. self-loops) are grouped by destination 128-block, padded to a
  tile count shared across cores (single SPMD program, per-core data).
- dma_gather fetches, per edge: the source node's value row (bf16, 512B) and
  alpha row (fp32, 256B); dst-side alpha rows come from a core-local table
  indexed by local dst id.  Source tables are split at gid 32768 (A/B halves)
  because gather indices are int16.
- e = exp(leaky_relu(alpha_src + alpha_dst)) per edge (DVE/ACT, batched).
- Segment softmax + aggregation via indicator matmuls on the tensor engine:
  S[t, j] = (dst_local[t] == j); denom += S^T @ e; agg += S^T @ (V ⊙ e),
  accumulated in PSUM per dst-block; normalized by 1/denom at flush.
  (The softmax max-subtraction is skipped: logits are O(5), exp is safe in
  fp32, and the normalized alphas are mathematically identical.)
- Layer 0 flush: +b0, LayerNorm, ELU -> z; layer 1 aggregates z directly and
  projects 256->2 at flush ((sum a z) @ W1 == sum a (z @ W1) by linearity).

gids 0 and 32768 are reserved as zero-pad nodes (x column forced to 0) so
out-of-half gather slots read exact zeros; the core-local alpha table for
layer 0 is then built as gather(lo-half) + gather(hi-half).
"""

import hashlib
import numpy as np

import concourse.bass as bass
import concourse.tile as tile
from concourse import bacc, mybir
from concourse.bass_utils import run_bass_kernel_spmd

F32 = mybir.dt.float32
BF16 = mybir.dt.bfloat16
I16 = mybir.dt.int16
ALU = mybir.AluOpType
ACTF = mybir.ActivationFunctionType


class CFG:
    n_nodes = 50000
    in_ch = 128
    hid = 64
    heads = 4
    out_ch = 2
    neg = 0.2
    eps = 1e-5
    n_cores = 8
    split = 32768
    gb = 3                   # dst-blocks per gather group
    val_w = 256              # value row width (elements)
    al_w = 64                # alpha row width in fp32 (=256B rows)

    def __init__(self, n_nodes=50000, split=32768, gb=2):
        self.n_nodes = n_nodes
        self.split = split
        self.gb = gb
        self.pc = n_nodes // self.n_cores
        self.bpc = (self.pc + 127) // 128
        self.pcpad = self.bpc * 128
        self.npad = self.n_cores * self.pcpad


cfg = CFG()


def configure(**kw):
    """Override module config (used by the test harness for mini runs)."""
    global cfg
    cfg = CFG(**kw)
    _cache.clear()


# ------------------------------------------------------------------ host ----

def _wrap_idx(idx):
    """[n] -> [128, n//16] int16: slot i at [i%16, i//16], replicated x8."""
    idx = np.asarray(idx, np.int16)
    n = idx.shape[0]
    assert n % 16 == 0
    w = idx.reshape(n // 16, 16).T
    return np.tile(w, (8, 1)).copy()


def _gid_map(c):
    """[n_nodes] -> padded gid; gids 0 and c.split are reserved pads."""
    gids = np.zeros(c.n_nodes, np.int64)
    for ci in range(c.n_cores):
        base = ci * c.pcpad
        slots = np.arange(c.pcpad)
        forb = []
        if base <= 0 < base + c.pcpad:
            forb.append(0 - base)
        if base <= c.split < base + c.pcpad:
            forb.append(c.split - base)
        if forb:
            keep = np.ones(c.pcpad, bool)
            keep[forb] = False
            slots = slots[keep]
        gids[ci * c.pc:(ci + 1) * c.pc] = base + slots[:c.pc]
    return gids


def prep(x, edge_index, W0, a_src0, a_dst0, b0, ln_g, ln_b, W1,
         a_src1, a_dst1, b1):
    c = cfg
    N, C, H, HID = c.n_nodes, c.n_cores, c.heads, c.hid
    x = np.asarray(x, np.float32)
    ei = np.asarray(edge_index, np.int64)
    loops = np.arange(N, dtype=np.int64)
    src = np.concatenate([ei[0], loops])
    dst = np.concatenate([ei[1], loops])

    gmap = _gid_map(c)
    gsrc = gmap[src]
    gdst = gmap[dst]
    core = gdst // c.pcpad
    loc = gdst % c.pcpad
    blk = loc // 128
    dstloc = loc % 128
    isB = (gsrc >= c.split).astype(np.int64)

    order = np.lexsort((gsrc, isB, blk, core))
    gsrc, core, blk, dstloc, loc, isB = (
        a[order] for a in (gsrc, core, blk, dstloc, loc, isB))

    counts = np.zeros((C, c.bpc, 2), np.int64)
    np.add.at(counts, (core, blk, isB), 1)
    ntiles = -(-counts // 128)
    nA = ntiles[:, :, 0].max(axis=0)
    nB = ntiles[:, :, 1].max(axis=0)
    TA, TB = int(nA.sum()), int(nB.sum())
    T = TA + TB
    offA = np.concatenate([[0], np.cumsum(nA)]).astype(np.int64)
    offB = np.concatenate([[0], np.cumsum(nB)]).astype(np.int64)

    idxA = np.zeros((C, max(TA, 1) * 128), np.int64)
    idxB = np.zeros((C, max(TB, 1) * 128), np.int64)
    dlA = np.full((C, max(TA, 1) * 128), -1.0, np.float32)
    dlB = np.full((C, max(TB, 1) * 128), -1.0, np.float32)
    locA = np.zeros((C, max(TA, 1) * 128), np.int64)
    locB = np.zeros((C, max(TB, 1) * 128), np.int64)

    starts = np.concatenate([[0], np.cumsum(counts.reshape(-1))])[:-1]
    starts = starts.reshape(C, c.bpc, 2)
    for ci in range(C):
        for b in range(c.bpc):
            n0 = counts[ci, b, 0]
            s0 = starts[ci, b, 0]
            a0 = offA[b] * 128
            idxA[ci, a0:a0 + n0] = gsrc[s0:s0 + n0]
            dlA[ci, a0:a0 + n0] = dstloc[s0:s0 + n0]
            locA[ci, a0:a0 + n0] = loc[s0:s0 + n0]
            locA[ci, a0 + n0:(offA[b] + nA[b]) * 128] = b * 128
            n1 = counts[ci, b, 1]
            s1 = starts[ci, b, 1]
            b0_ = offB[b] * 128
            idxB[ci, b0_:b0_ + n1] = gsrc[s1:s1 + n1] - c.split
            dlB[ci, b0_:b0_ + n1] = dstloc[s1:s1 + n1]
            locB[ci, b0_:b0_ + n1] = loc[s1:s1 + n1]
            locB[ci, b0_ + n1:(offB[b] + nB[b]) * 128] = b * 128

    # consumption-order arrays: per block, A tiles then B tiles
    dst_cols = np.zeros((C, 128, max(T, 1)), np.float32)
    idxD = np.zeros((C, max(T, 1) * 128), np.int64)
    tpos = 0
    for b in range(c.bpc):
        na, nb = int(nA[b]), int(nB[b])
        a0, b0_ = offA[b] * 128, offB[b] * 128
        for ci in range(C):
            seg = np.concatenate(
                [dlA[ci, a0:a0 + na * 128], dlB[ci, b0_:b0_ + nb * 128]])
            dst_cols[ci, :, tpos:tpos + na + nb] = seg.reshape(na + nb, 128).T
            idxD[ci, tpos * 128:(tpos + na + nb) * 128] = np.concatenate(
                [locA[ci, a0:a0 + na * 128], locB[ci, b0_:b0_ + nb * 128]])
        tpos += na + nb

    # own-gid index lists for the adloc0 build (lo/hi halves; pads -> row 0)
    own = np.zeros((C, c.pcpad), np.int64)
    for ci in range(C):
        own[ci] = ci * c.pcpad + np.arange(c.pcpad)
    own_lo = np.where(own < c.split, own, 0)
    own_hi = np.where(own >= c.split, own - c.split, 0)

    per_core = []
    for ci in range(C):
        per_core.append(dict(
            idxA=_wrap_idx(idxA[ci]),
            idxB=_wrap_idx(idxB[ci]),
            idxD=_wrap_idx(idxD[ci]),
            idxOlo=_wrap_idx(own_lo[ci]),
            idxOhi=_wrap_idx(own_hi[ci]),
            dst_cols=np.ascontiguousarray(dst_cols[ci]),
        ))

    xT = np.zeros((c.in_ch, c.npad), np.float32)
    xT[:, gmap] = x.T

    W0 = np.asarray(W0, np.float32)
    As = np.einsum("ihc,hc->ih", W0.reshape(c.in_ch, H, HID),
                   np.asarray(a_src0, np.float32))
    Ad = np.einsum("ihc,hc->ih", W0.reshape(c.in_ch, H, HID),
                   np.asarray(a_dst0, np.float32))
    wpack = np.concatenate([W0, As, Ad], axis=1).astype(np.float32)

    W1 = np.asarray(W1, np.float32)
    ws = (W1 @ np.asarray(a_src1, np.float32)[0]).astype(np.float32)
    wd = (W1 @ np.asarray(a_dst1, np.float32)[0]).astype(np.float32)

    def rep(v):
        v = np.asarray(v, np.float32)
        return np.broadcast_to(v[None, :], (128, v.shape[0])).copy()

    shared = dict(
        xT=xT, wpack=wpack, w1=W1.astype(np.float32),
        b0rep=rep(b0), grep=rep(ln_g), brep=rep(ln_b),
        wsrep=rep(ws), wdrep=rep(wd), b1rep=rep(np.asarray(b1, np.float32)),
        iota=np.broadcast_to(
            np.arange(128, dtype=np.float32)[None, :], (128, 128)).copy(),
        ident=np.eye(128, dtype=np.float32),
    )
    sched = dict(nA=nA, nB=nB, offA=offA, offB=offB, TA=TA, TB=TB, T=T)
    return sched, shared, per_core, gmap


# ----------------------------------------------------------------- build ----

def build(sched, stage="full", sub=99, repeat=1):
    """stage: one of ph0, adloc, l0, ag, full — truncate program for bisect.
    sub: 1=gathers 2=+e 3=+vscale 4=+matmuls 99=full (within emit_layer).
    repeat: emit the whole body N times (device-time delta measurement)."""
    global _SUB
    _SUB = sub
    c = cfg
    H, VW, AW = c.heads, c.val_w, c.al_w
    nA, nB = sched["nA"], sched["nB"]
    offA, offB = sched["offA"], sched["offB"]
    TA, TB, T = sched["TA"], sched["TB"], sched["T"]

    nc = bacc.Bacc("TRN2", target_bir_lowering=False, debug=False,
                   num_devices=c.n_cores,
                   num_swdge_queues=4, dynamic_dma_scratch_size=65536)
    VR = VW + 128            # merged value-row width in bf16 (768B):
    AC = VW                  # [z bf16 0:256 | as/ad fp32 bit-packed 256:272]

    def inp(name, shape, dt):
        return nc.dram_tensor(name, list(shape), dt, kind="ExternalInput")

    xT_d = inp("xT", (c.in_ch, c.npad), F32)
    wpack_d = inp("wpack", (c.in_ch, VW + 2 * H), F32)
    w1_d = inp("w1", (VW, c.out_ch), F32)
    b0_d = inp("b0rep", (128, VW), F32)
    g_d = inp("grep", (128, VW), F32)
    bln_d = inp("brep", (128, VW), F32)
    ws_d = inp("wsrep", (128, VW), F32)
    wd_d = inp("wdrep", (128, VW), F32)
    b1_d = inp("b1rep", (128, c.out_ch), F32)
    iota_d = inp("iota", (128, 128), F32)
    ident_d = inp("ident", (128, 128), F32)
    idxA_d = inp("idxA", (128, max(TA, 1) * 8), I16)
    idxB_d = inp("idxB", (128, max(TB, 1) * 8), I16)
    idxD_d = inp("idxD", (128, max(T, 1) * 8), I16)
    idxOlo_d = inp("idxOlo", (128, c.pcpad // 16), I16)
    idxOhi_d = inp("idxOhi", (128, c.pcpad // 16), I16)
    dst_d = inp("dst_cols", (128, max(T, 1)), F32)

    val0 = nc.dram_tensor("val0", [c.npad, VR], BF16)
    adloc0 = nc.dram_tensor("adloc0", [c.pcpad, AW], F32)
    val1loc = nc.dram_tensor("val1loc", [c.pcpad, VR], BF16)
    adloc1 = nc.dram_tensor("adloc1", [c.pcpad, AW], F32)
    val1 = nc.dram_tensor("val1", [c.npad, VR], BF16, addr_space="Shared")
    outp = nc.dram_tensor("outp", [c.pcpad, c.out_ch], F32,
                          kind="ExternalOutput")
    dbg = nc.dram_tensor("dbg", [128, 4096], F32)

    groups = list(range(0, c.bpc, c.gb))

    with tile.TileContext(nc) as tc:
        with (
            tc.tile_pool(name="const", bufs=1) as cpool,
            tc.tile_pool(name="xchunk", bufs=2) as xpool,
            tc.tile_pool(name="ph0", bufs=2) as p0pool,
            tc.tile_pool(name="psmm", bufs=2, space="PSUM") as psmm,
            tc.tile_pool(name="gath", bufs=2) as gpool,
            tc.tile_pool(name="idx", bufs=2) as ipool,
            tc.tile_pool(name="work", bufs=2) as wpool,
            tc.tile_pool(name="psagg", bufs=2, space="PSUM") as psagg,
            tc.tile_pool(name="psden", bufs=2, space="PSUM") as psden,
            tc.tile_pool(name="pstp", bufs=1, space="PSUM") as pstp,
            tc.tile_pool(name="psproj", bufs=1, space="PSUM") as psproj,
            tc.tile_pool(name="flush", bufs=2) as fpool,
        ):
            def cload(ap, shape, dt, tag):
                t = cpool.tile(shape, dt, tag=tag)
                nc.sync.dma_start(t[:], ap[:])
                return t

            wpack_t = cload(wpack_d, [c.in_ch, VW + 2 * H], F32, "wpack")
            b0_t = cload(b0_d, [128, VW], F32, "b0")
            g_t = cload(g_d, [128, VW], F32, "g")
            bln_t = cload(bln_d, [128, VW], F32, "bln")
            ws_t = cload(ws_d, [128, VW], F32, "ws")
            wd_t = cload(wd_d, [128, VW], F32, "wd")
            b1_t = cload(b1_d, [128, c.out_ch], F32, "b1")
            ident_t = cload(ident_d, [128, 128], F32, "ident")
            dst_t = cload(dst_d, [128, max(T, 1)], F32, "dstc")
            iota_t = cload(iota_d, [128, 128], F32, "iota")
            w1a = cpool.tile([128, c.out_ch], F32, tag="w1a")
            nc.sync.dma_start(w1a[:], w1_d[0:128, :])
            w1b = cpool.tile([128, c.out_ch], F32, tag="w1b")
            nc.sync.dma_start(w1b[:], w1_d[128:256, :])

            def emit_body():
                # ---------------- phase 0: node transform for all gids ----------
                # staging holds 4 blocks' merged rows; one table DMA per chunk
                nblk_tot = c.n_cores * c.bpc
                for bg in range(0, nblk_tot, 4):
                    nbk = min(4, nblk_tot - bg)
                    xc = xpool.tile([c.in_ch, 4 * 128], F32, tag="xc")
                    nc.sync.dma_start(
                        xc[:, :nbk * 128], xT_d[:, bg * 128:(bg + nbk) * 128])
                    hz = p0pool.tile([128, 4, VR], BF16, tag="hz")
                    for k in range(nbk):
                        ps = psmm.tile([128, VW + 2 * H], F32, tag="ph0ps")
                        nc.tensor.matmul(ps[:], xc[:, k * 128:(k + 1) * 128],
                                         wpack_t[:], start=True, stop=True)
                        nc.scalar.activation(hz[:, k, 0:VW], ps[:, 0:VW],
                                             ACTF.Copy)
                        nc.vector.tensor_copy(
                            hz[:, k, AC:AC + 4 * H].bitcast(F32),
                            ps[:, VW:VW + 2 * H])
                    # val0 rows bg*128 .. (bg+nbk)*128 from [128, nbk, VR]
                    dst_view = val0[bg * 128:(bg + nbk) * 128, 0:AC + 4 * H]\
                        .rearrange("(k p) w -> p k w", p=128)
                    nc.sync.dma_start(dst_view, hz[:, :nbk, 0:AC + 4 * H])

                # ---------------- adloc0: own rows of al0, via lo+hi gathers ----
                _order = ["ph0", "adloc", "l0", "ag", "full"]
                _lvl = _order.index(stage)
                ntile_own = c.pcpad // 128
                if _lvl >= 1:
                  with tc.tile_pool(name="adbuild", bufs=1) as apool:
                      olo_t = apool.tile([128, c.pcpad // 16], I16, tag="olo")
                      nc.sync.dma_start(olo_t[:], idxOlo_d[:])
                      ohi_t = apool.tile([128, c.pcpad // 16], I16, tag="ohi")
                      nc.sync.dma_start(ohi_t[:], idxOhi_d[:])
                      gsum = apool.tile([128, ntile_own, 2 * H], F32, tag="gsum")
                      CH = 8  # tiles per chunk
                      for t0 in range(0, ntile_own, CH):
                          t1 = min(t0 + CH, ntile_own)
                          nt = t1 - t0
                          glo = apool.tile([128, CH, VR], BF16, tag="glo")
                          nc.gpsimd.dma_gather(
                              out_ap=glo[:, :nt, :], in_ap=val0[0:c.split, :],
                              idxs_ap=olo_t[:, t0 * 8:t1 * 8],
                              num_idxs=nt * 128,
                              num_idxs_reg=nt * 128, elem_size=VR,
                              single_packet=False, queue_num=0)
                          ghi = apool.tile([128, CH, VR], BF16, tag="ghi")
                          nc.gpsimd.dma_gather(
                              out_ap=ghi[:, :nt, :],
                              in_ap=val0[c.split:c.npad, :],
                              idxs_ap=ohi_t[:, t0 * 8:t1 * 8],
                              num_idxs=nt * 128,
                              num_idxs_reg=nt * 128, elem_size=VR,
                              single_packet=False, queue_num=1)
                          nc.vector.tensor_tensor(
                              gsum[:, t0:t1, :],
                              glo[:, :nt, AC:AC + 4 * H].bitcast(F32),
                              ghi[:, :nt, AC:AC + 4 * H].bitcast(F32),
                              ALU.add)
                      # slot i lives at partition i%128, free slot i//128
                      dstv = adloc0[:, 0:2 * H].rearrange(
                          "(t p) w -> p t w", p=128)
                      nc.sync.dma_start(dstv, gsum[:])

                # ---------------- one attention layer ---------------------------
                def emit_layer(lyr):
                    HL = H if lyr == 0 else 1
                    valt = val0 if lyr == 0 else val1
                    adl = adloc0 if lyr == 0 else adloc1
                    for g0 in groups:
                        g1 = min(g0 + c.gb, c.bpc)
                        blks = range(g0, g1)
                        tA0, tA1 = int(offA[g0]), int(offA[g1])
                        tB0, tB1 = int(offB[g0]), int(offB[g1])
                        nAg, nBg = tA1 - tA0, tB1 - tB0
                        nG = nAg + nBg
                        if nG == 0:
                            continue
                        # idx slices
                        if nAg:
                            iA = ipool.tile([128, nAg * 8], I16, tag="iA")
                            nc.sync.dma_start(iA[:], idxA_d[:, tA0 * 8:tA1 * 8])
                        if nBg:
                            iB = ipool.tile([128, nBg * 8], I16, tag="iB")
                            nc.sync.dma_start(iB[:], idxB_d[:, tB0 * 8:tB1 * 8])
                        tD0 = tA0 + tB0
                        iD = ipool.tile([128, nG * 8], I16, tag="iD")
                        nc.sync.dma_start(
                            iD[:], idxD_d[:, tD0 * 8:(tD0 + nG) * 8])

                        # gathers: merged value+alpha rows from A/B halves,
                        # alpha-dst from the core-local table
                        vA = gpool.tile([128, max(nAg, 1), VR], BF16, tag="vA")
                        if nAg:
                            nc.gpsimd.dma_gather(
                                out_ap=vA[:], in_ap=valt[0:c.split, :],
                                idxs_ap=iA[:], num_idxs=nAg * 128,
                                num_idxs_reg=nAg * 128, elem_size=VR,
                                single_packet=False, queue_num=0)
                        vB = gpool.tile([128, max(nBg, 1), VR], BF16, tag="vB")
                        if nBg:
                            nc.gpsimd.dma_gather(
                                out_ap=vB[:], in_ap=valt[c.split:c.npad, :],
                                idxs_ap=iB[:], num_idxs=nBg * 128,
                                num_idxs_reg=nBg * 128, elem_size=VR,
                                single_packet=False, queue_num=1)
                        aD = gpool.tile([128, nG, AW], F32, tag="aD")
                        nc.gpsimd.dma_gather(
                            out_ap=aD[:], in_ap=adl[:, :], idxs_ap=iD[:],
                            num_idxs=nG * 128, num_idxs_reg=nG * 128,
                            elem_size=AW, single_packet=False, queue_num=2)

                        if _SUB == 1:   # consume gathers, skip compute
                            nc.sync.dma_start(dbg[:, 0:AW], aD[:, 0, :])
                            if nAg:
                                cv = wpool.tile([128, VW], F32, tag="cv")
                                nc.vector.tensor_copy(cv[:], vA[:, 0, 0:VW])
                                nc.sync.dma_start(dbg[:, 512:512 + VW], cv[:])
                            continue

                        # e = exp(lrelu(as + ad)) for all slots of the group, in
                        # consumption order.  as comes from aA/aB per block; ad
                        # from aD (already consumption-ordered).  Compute per
                        # block segment to keep slot order aligned.
                        e_t = wpool.tile([128, nG, HL], BF16, tag="e")
                        logit = wpool.tile([128, nG, HL], F32, tag="logit")
                        pos = 0
                        for b in blks:
                            na, nb_ = int(nA[b]), int(nB[b])
                            if na:
                                sA = int(offA[b]) - tA0
                                asv = vA[:, sA:sA + na, AC:AC + 4 * H]\
                                    .bitcast(F32)
                                nc.vector.tensor_tensor(
                                    logit[:, pos:pos + na, :],
                                    asv[:, :, 0:HL],
                                    aD[:, pos:pos + na, H:H + HL],
                                    ALU.add)
                                pos += na
                            if nb_:
                                sB = int(offB[b]) - tB0
                                bsv = vB[:, sB:sB + nb_, AC:AC + 4 * H]\
                                    .bitcast(F32)
                                nc.vector.tensor_tensor(
                                    logit[:, pos:pos + nb_, :],
                                    bsv[:, :, 0:HL],
                                    aD[:, pos:pos + nb_, H:H + HL],
                                    ALU.add)
                                pos += nb_
                        lr = wpool.tile([128, nG, HL], F32, tag="lr")
                        nc.vector.scalar_tensor_tensor(
                            lr[:], logit[:], float(c.neg), logit[:],
                            ALU.mult, ALU.max)
                        nc.scalar.activation(e_t[:], lr[:], ACTF.Exp)

                        if _SUB == 2:   # stop after e chain
                            ce = wpool.tile([128, nG], F32, tag="ce")
                            nc.vector.tensor_copy(ce[:], e_t[:, :, 0])
                            nc.sync.dma_start(dbg[:, 0:nG], ce[:])
                            continue

                        # V' = V * e (per head), in A/B stream order
                        pos = 0
                        for b in blks:
                            na, nb_ = int(nA[b]), int(nB[b])
                            if na:
                                sA = int(offA[b]) - tA0
                                vv = vA[:, sA:sA + na, 0:VW].rearrange(
                                    "p t (h c) -> p t h c", h=HL)
                                ee = e_t[:, pos:pos + na, :].unsqueeze(3)
                                nc.vector.tensor_tensor(
                                    vv, vv, ee.broadcast_to(
                                        [128, na, HL, VW // HL]), ALU.mult)
                                pos += na
                            if nb_:
                                sB = int(offB[b]) - tB0
                                vv = vB[:, sB:sB + nb_, 0:VW].rearrange(
                                    "p t (h c) -> p t h c", h=HL)
                                ee = e_t[:, pos:pos + nb_, :].unsqueeze(3)
                                nc.vector.tensor_tensor(
                                    vv, vv, ee.broadcast_to(
                                        [128, nb_, HL, VW // HL]), ALU.mult)
                                pos += nb_

                        if _SUB == 3:   # stop after V'-scale
                            cv = wpool.tile([128, VW], F32, tag="cv")
                            nc.vector.tensor_copy(cv[:], vA[:, 0, :])
                            nc.sync.dma_start(dbg[:, 0:VW], cv[:])
                            continue

                        # per block: S build, matmul accumulate, flush
                        pos = 0
                        for b in blks:
                            na, nb_ = int(nA[b]), int(nB[b])
                            tb = na + nb_
                            if tb == 0:
                                continue
                            tcol0 = tD0 + pos
                            s_t = wpool.tile([128, tb * 128], BF16, tag="S")
                            nc.vector.tensor_tensor(
                                s_t[:].rearrange("p (t j) -> p t j", j=128),
                                dst_t[:, tcol0:tcol0 + tb].unsqueeze(2)
                                .broadcast_to([128, tb, 128]),
                                iota_t[:].unsqueeze(1).broadcast_to([128, tb, 128]),
                                ALU.is_equal)
                            agg = psagg.tile([128, VW], F32, tag="agg")
                            den = psden.tile([128, HL], F32, tag="den")
                            for t in range(tb):
                                lhs = s_t[:, t * 128:(t + 1) * 128]
                                if t < na:
                                    vv = vA[:, int(offA[b]) - tA0 + t, 0:VW]
                                else:
                                    vv = vB[:, int(offB[b]) - tB0 + (t - na),
                                            0:VW]
                                st_, sp_ = (t == 0), (t == tb - 1)
                                nc.tensor.matmul(agg[:], lhs, vv,
                                                 start=st_, stop=sp_)
                                nc.tensor.matmul(den[:], lhs,
                                                 e_t[:, pos + t, :],
                                                 start=st_, stop=sp_)
                            pos += tb

                            if _SUB == 4:   # stop after matmuls
                                ca = fpool.tile([128, VW], F32, tag="ca")
                                nc.vector.tensor_copy(ca[:], agg[:])
                                nc.sync.dma_start(dbg[:, 0:VW], ca[:])
                                continue

                            # ---- flush this block
                            # +1e-30 keeps pad rows (denom 0) finite; for real
                            # rows (denom > 1e-3) it is far below fp32 ulp.
                            deneps = fpool.tile([128, HL], F32, tag="deneps")
                            nc.vector.tensor_scalar_add(deneps[:], den[:], 1e-30)
                            rcp = fpool.tile([128, HL], F32, tag="rcp")
                            nc.vector.reciprocal(rcp[:], deneps[:])
                            sc = fpool.tile([128, VW], F32, tag="sc")
                            nc.vector.tensor_tensor(
                                sc[:].rearrange("p (h c) -> p h c", h=HL),
                                agg[:].rearrange("p (h c) -> p h c", h=HL),
                                rcp[:].unsqueeze(2).broadcast_to(
                                    [128, HL, VW // HL]),
                                ALU.mult)
                            if _SUB == 5:   # stop after rcp-scale
                                nc.sync.dma_start(dbg[:, 0:VW], sc[:])
                                continue
                            rows = slice(b * 128, (b + 1) * 128)
                            if lyr == 0:
                                flush0(sc, rows)
                            else:
                                flush1(sc, rows)

                # ---- layer-0 flush: +b0, LayerNorm, ELU, z tables --------------
                def flush0(sc, rows):
                    nc.vector.tensor_tensor(sc[:], sc[:], b0_t[:], ALU.add)
                    if _SUB == 61:
                        nc.sync.dma_start(dbg[:, 0:VW], sc[:])
                        return
                    mu = fpool.tile([128, 1], F32, tag="mu")
                    nc.vector.tensor_reduce(
                        mu[:], sc[:], mybir.AxisListType.X, ALU.add)
                    nc.vector.tensor_scalar_mul(mu[:], mu[:], 1.0 / VW)
                    if _SUB == 62:
                        nc.sync.dma_start(dbg[:, 0:1], mu[:])
                        return
                    xc_ = fpool.tile([128, VW], F32, tag="xc0")
                    nc.vector.tensor_scalar(
                        xc_[:], sc[:], mu[:], None, ALU.subtract)
                    if _SUB == 63:
                        nc.sync.dma_start(dbg[:, 0:VW], xc_[:])
                        return
                    sq = fpool.tile([128, VW], F32, tag="sq")
                    nc.vector.tensor_tensor(sq[:], xc_[:], xc_[:], ALU.mult)
                    var = fpool.tile([128, 1], F32, tag="var")
                    nc.vector.tensor_reduce(
                        var[:], sq[:], mybir.AxisListType.X, ALU.add)
                    nc.vector.tensor_scalar(
                        var[:], var[:], 1.0 / VW, float(c.eps), ALU.mult, ALU.add)
                    if _SUB == 64:
                        nc.sync.dma_start(dbg[:, 0:1], var[:])
                        return
                    sd = fpool.tile([128, 1], F32, tag="sd")
                    nc.scalar.activation(sd[:], var[:], ACTF.Sqrt)
                    rstd = fpool.tile([128, 1], F32, tag="rstd")
                    nc.vector.reciprocal(rstd[:], sd[:])
                    if _SUB == 65:
                        nc.sync.dma_start(dbg[:, 0:1], rstd[:])
                        return
                    zz = fpool.tile([128, VW], F32, tag="zz")
                    nc.vector.scalar_tensor_tensor(
                        zz[:], xc_[:], rstd[:], g_t[:], ALU.mult, ALU.mult)
                    nc.vector.tensor_tensor(zz[:], zz[:], bln_t[:], ALU.add)
                    if _SUB == 6:   # stop after LN
                        nc.sync.dma_start(dbg[:, 0:VW], zz[:])
                        return
                    # ELU: z = max(x,0) + exp(min(x,0)) - 1
                    zmin = fpool.tile([128, VW], F32, tag="zmin")
                    nc.vector.tensor_scalar_min(zmin[:], zz[:], 0.0)
                    pexp = fpool.tile([128, VW], F32, tag="pexp")
                    nc.scalar.activation(pexp[:], zmin[:], ACTF.Exp)
                    zmax = fpool.tile([128, VW], F32, tag="zmax")
                    nc.vector.tensor_scalar_max(zmax[:], zz[:], 0.0)
                    z = fpool.tile([128, VW], F32, tag="z")
                    nc.vector.tensor_tensor(z[:], zmax[:], pexp[:], ALU.add)
                    nc.vector.tensor_scalar_add(z[:], z[:], -1.0)
                    if _SUB == 7:   # stop after ELU
                        nc.sync.dma_start(dbg[:, 0:VW], z[:])
                        return
                    # attention scalars for layer 1
                    scr = fpool.tile([128, VW], F32, tag="scr")
                    a1 = fpool.tile([128, 2 * H], F32, tag="a1")
                    nc.gpsimd.memset(a1[:], 0.0)
                    nc.vector.tensor_tensor(scr[:], z[:], ws_t[:], ALU.mult)
                    nc.vector.tensor_reduce(
                        a1[:, 0:1], scr[:], mybir.AxisListType.X, ALU.add)
                    nc.vector.tensor_tensor(scr[:], z[:], wd_t[:], ALU.mult)
                    nc.vector.tensor_reduce(
                        a1[:, H:H + 1], scr[:], mybir.AxisListType.X, ALU.add)
                    zb = fpool.tile([128, VW], BF16, tag="zb")
                    nc.scalar.activation(zb[:], z[:], ACTF.Copy)
                    nc.sync.dma_start(val1loc[rows, 0:VW], zb[:])
                    nc.sync.dma_start(
                        val1loc[rows, AC:AC + 4 * H].bitcast(F32), a1[:])
                    nc.sync.dma_start(adloc1[rows, 0:2 * H], a1[:])

                # ---- layer-1 flush: project 256->2, +b1, store -----------------
                def flush1(sc, rows):
                    po = psproj.tile([128, c.out_ch], F32, tag="proj")
                    for k2 in range(2):
                        tp = pstp.tile([128, 128], F32, tag="tp")
                        nc.tensor.transpose(
                            tp[:], sc[:, k2 * 128:(k2 + 1) * 128], ident_t[:])
                        tps = fpool.tile([128, 128], F32, tag="tps")
                        nc.scalar.activation(tps[:], tp[:], ACTF.Copy)
                        nc.tensor.matmul(po[:], tps[:], (w1a if k2 == 0 else w1b)[:],
                                         start=(k2 == 0), stop=(k2 == 1))
                    ob = fpool.tile([128, c.out_ch], F32, tag="ob")
                    nc.vector.tensor_tensor(ob[:], po[:], b1_t[:], ALU.add)
                    nc.sync.dma_start(outp[rows, :], ob[:])

                if _lvl >= 2:
                    emit_layer(0)

                if _lvl >= 3:
                    # ---- allgather z tables ------------------------------------
                    nc.gpsimd.collective_compute(
                        "AllGather", ALU.bypass,
                        replica_groups=[list(range(c.n_cores))],
                        ins=[val1loc[:]], outs=[val1[:]])

                if _lvl >= 4:
                    emit_layer(1)

            for _rep in range(repeat):
                emit_body()

    nc.compile()
    return nc


# ------------------------------------------------------------------ run -----

_cache = {}
_SUB = 99


def kernel(**inputs):
    c = cfg
    sched, shared, per_core, gmap = prep(**inputs)
    key = _sched_sig(sched)
    if key in _cache:
        nc = _cache[key]
    else:
        nc = build(sched)
        _cache[key] = nc

    in_maps = []
    for ci in range(c.n_cores):
        m = dict(shared)
        m.update(per_core[ci])
        in_maps.append(m)
    res = run_bass_kernel_spmd(nc, in_maps, list(range(c.n_cores)))
    out = np.zeros((c.n_nodes, c.out_ch), np.float32)
    loc = gmap % c.pcpad
    for ci in range(c.n_cores):
        sel = slice(ci * c.pc, (ci + 1) * c.pc)
        out[sel] = res.results[ci]["outp"][loc[sel]]
    return out.astype(np.float32)


def _sched_sig(s):
    return (tuple(s["nA"].tolist()), tuple(s["nB"].tolist()))



# revision 16
# speedup vs baseline: 1.1754x; 1.1754x over previous
"""Two-layer GAT (GATConv(128->4x64, concat) + LayerNorm + ELU +
GATConv(256->2)) on 8 trn2 NeuronCores via Bass/Tile.

Distribution (graph/data parallel): destination nodes are partitioned across
8 cores; weights replicated; each core computes the dense node transform for
all nodes (cheap); layer-1 value tables are AllGathered at the layer boundary.

v2 design (vs baseline): per-edge gathers are row-count-bound (~7ns/row on
SWDGE regardless of row bytes), so the per-edge dst-alpha gather is replaced
by an on-chip broadcast: St[j, slot] = (dst[slot] == j) built by DVE is_equal
from a host-shipped replicated dst row (bf16), then ad_slot = St^T @ adblock
on the tensor engine.  Dst-alpha tables live in SBUF (no DRAM round trip).
Layer-1 values are pre-projected in the layer-0 flush (out = sum a (z@W1)),
shrinking layer-1 gather rows to 256B and its flush to a reciprocal.
The softmax denominator is folded into the aggregation matmul as extra rhs
columns (e is written into each gathered row's tail slot).
LayerNorm/ELU run mostly on ACT (Identity-with-bias / Square-accum / Sqrt /
Relu / Exp) with remaining DVE ops in non-port-contending forms
(tensor_tensor / scalar_tensor_tensor / tensor_reduce only), so SWDGE gather
descriptor generation is never starved by DVE perf-mode port locks.

Row layouts (bf16 tables):
  val0 [npad, 384]: z 0:256 | as+ad fp32 bit-packed 256:272 | e slot 272:276
  val1 [npad, 128]: zW1 0:2 | (0) 2:3 | e slot 3:4 | as1 fp32 bit-packed 4:6
gids 0 and 32768 are reserved zero-pad nodes so out-of-half gather slots read
exact zeros (gather indices are int16, tables split at 32768).
"""

import numpy as np
import ml_dtypes

import concourse.bass as bass
import concourse.tile as tile
from concourse import bacc, mybir
from concourse.bass_utils import run_bass_kernel_spmd

F32 = mybir.dt.float32
BF16 = mybir.dt.bfloat16
I16 = mybir.dt.int16
ALU = mybir.AluOpType
ACTF = mybir.ActivationFunctionType
BF = ml_dtypes.bfloat16


class CFG:
    n_nodes = 50000
    in_ch = 128
    hid = 64
    heads = 4
    out_ch = 2
    neg = 0.2
    eps = 1e-5
    n_cores = 8
    split = 32768
    gb = 2                   # dst-blocks per gather group
    val_w = 256              # z row width (elements)

    def __init__(self, n_nodes=50000, split=32768, gb=2):
        self.n_nodes = n_nodes
        self.split = split
        self.gb = gb
        self.pc = n_nodes // self.n_cores
        self.bpc = (self.pc + 127) // 128
        self.pcpad = self.bpc * 128
        self.npad = self.n_cores * self.pcpad


cfg = CFG()


def configure(**kw):
    global cfg
    cfg = CFG(**kw)
    _cache.clear()


# ------------------------------------------------------------------ host ----

def _wrap_idx(idx):
    """[n] -> [128, n//16] int16: slot i at [i%16, i//16], replicated x8."""
    idx = np.asarray(idx, np.int16)
    n = idx.shape[0]
    assert n % 16 == 0
    w = idx.reshape(n // 16, 16).T
    return np.tile(w, (8, 1)).copy()


def _gid_map(c):
    """[n_nodes] -> padded gid; gids 0 and c.split are reserved pads."""
    gids = np.zeros(c.n_nodes, np.int64)
    for ci in range(c.n_cores):
        base = ci * c.pcpad
        slots = np.arange(c.pcpad)
        forb = []
        if base <= 0 < base + c.pcpad:
            forb.append(0 - base)
        if base <= c.split < base + c.pcpad:
            forb.append(c.split - base)
        if forb:
            keep = np.ones(c.pcpad, bool)
            keep[forb] = False
            slots = slots[keep]
        gids[ci * c.pc:(ci + 1) * c.pc] = base + slots[:c.pc]
    return gids


def prep(x, edge_index, W0, a_src0, a_dst0, b0, ln_g, ln_b, W1,
         a_src1, a_dst1, b1):
    c = cfg
    N, C, H = c.n_nodes, c.n_cores, c.heads
    x = np.asarray(x, np.float32)
    ei = np.asarray(edge_index, np.int64)
    loops = np.arange(N, dtype=np.int64)
    src = np.concatenate([ei[0], loops])
    dst = np.concatenate([ei[1], loops])

    gmap = _gid_map(c)
    gsrc = gmap[src]
    gdst = gmap[dst]
    core = gdst // c.pcpad
    loc = gdst % c.pcpad
    blk = loc // 128
    dstloc = loc % 128
    isB = (gsrc >= c.split).astype(np.int64)

    order = np.lexsort((gsrc, isB, blk, core))
    gsrc, core, blk, dstloc, isB = (
        a[order] for a in (gsrc, core, blk, dstloc, isB))

    counts = np.zeros((C, c.bpc, 2), np.int64)
    np.add.at(counts, (core, blk, isB), 1)
    ntiles = -(-counts // 128)
    nA = ntiles[:, :, 0].max(axis=0)
    nB = ntiles[:, :, 1].max(axis=0)
    TA, TB = int(nA.sum()), int(nB.sum())
    T = TA + TB
    offA = np.concatenate([[0], np.cumsum(nA)]).astype(np.int64)
    offB = np.concatenate([[0], np.cumsum(nB)]).astype(np.int64)

    idxA = np.zeros((C, max(TA, 1) * 128), np.int64)
    idxB = np.zeros((C, max(TB, 1) * 128), np.int64)
    dlA = np.full((C, max(TA, 1) * 128), -1.0, np.float32)
    dlB = np.full((C, max(TB, 1) * 128), -1.0, np.float32)

    starts = np.concatenate([[0], np.cumsum(counts.reshape(-1))])[:-1]
    starts = starts.reshape(C, c.bpc, 2)
    for ci in range(C):
        for b in range(c.bpc):
            n0 = counts[ci, b, 0]
            s0 = starts[ci, b, 0]
            a0 = offA[b] * 128
            idxA[ci, a0:a0 + n0] = gsrc[s0:s0 + n0]
            dlA[ci, a0:a0 + n0] = dstloc[s0:s0 + n0]
            n1 = counts[ci, b, 1]
            s1 = starts[ci, b, 1]
            b0_ = offB[b] * 128
            idxB[ci, b0_:b0_ + n1] = gsrc[s1:s1 + n1] - c.split
            dlB[ci, b0_:b0_ + n1] = dstloc[s1:s1 + n1]

    # consumption-order arrays: per block, A tiles then B tiles
    dst_cols = np.zeros((C, 128, max(T, 1)), np.float32)
    dstT = np.full((C, max(T, 1) * 128), -1.0, np.float32)
    tpos = 0
    for b in range(c.bpc):
        na, nb = int(nA[b]), int(nB[b])
        a0, b0_ = offA[b] * 128, offB[b] * 128
        for ci in range(C):
            seg = np.concatenate(
                [dlA[ci, a0:a0 + na * 128], dlB[ci, b0_:b0_ + nb * 128]])
            dst_cols[ci, :, tpos:tpos + na + nb] = seg.reshape(na + nb, 128).T
            dstT[ci, tpos * 128:(tpos + na + nb) * 128] = seg
        tpos += na + nb

    per_core = []
    for ci in range(C):
        per_core.append(dict(
            idxA=_wrap_idx(idxA[ci]),
            idxB=_wrap_idx(idxB[ci]),
            dst_cols=np.ascontiguousarray(dst_cols[ci]),
            dstTb=np.broadcast_to(
                dstT[ci].astype(BF)[None, :], (128, dstT.shape[1])).copy(),
        ))

    # own-gid index lists for the ad0 build (lo/hi halves; pads -> row 0)
    own = np.zeros((C, c.pcpad), np.int64)
    for ci in range(C):
        own[ci] = ci * c.pcpad + np.arange(c.pcpad)
    own_lo = np.where(own < c.split, own, 0)
    own_hi = np.where(own >= c.split, own - c.split, 0)
    for ci in range(C):
        per_core[ci]["idxOlo"] = _wrap_idx(own_lo[ci])
        per_core[ci]["idxOhi"] = _wrap_idx(own_hi[ci])

    xT = np.zeros((c.in_ch, c.npad), np.float32)
    xT[:, gmap] = x.T

    W0 = np.asarray(W0, np.float32)
    As = np.einsum("ihc,hc->ih", W0.reshape(c.in_ch, H, c.hid),
                   np.asarray(a_src0, np.float32))
    Ad = np.einsum("ihc,hc->ih", W0.reshape(c.in_ch, H, c.hid),
                   np.asarray(a_dst0, np.float32))
    wpack = np.concatenate([W0, As, Ad], axis=1).astype(np.float32)

    W1 = np.asarray(W1, np.float32)
    ws = (W1 @ np.asarray(a_src1, np.float32)[0]).astype(np.float32)
    wd = (W1 @ np.asarray(a_dst1, np.float32)[0]).astype(np.float32)

    def rep(v):
        v = np.asarray(v, np.float32)
        return np.broadcast_to(v[None, :], (128, v.shape[0])).copy()

    misc = np.zeros((128, 4), np.float32)
    misc[:, 0] = 1e-30
    misc[:, 1] = 1.0
    misc[:, 3] = CFG.eps
    iotaP = np.broadcast_to(
        np.arange(128, dtype=np.float32)[:, None], (128, 1)).astype(BF).copy()

    shared = dict(
        xT=xT, wpack=wpack,
        b0rep=rep(b0), grep=rep(ln_g), brep=rep(ln_b),
        wsrep=rep(ws), wdrep=rep(wd),
        w1c0rep=rep(W1[:, 0]), w1c1rep=rep(W1[:, 1]),
        b1rep=rep(np.asarray(b1, np.float32)),
        misc=misc, iotaP=iotaP,
        iota=np.broadcast_to(
            np.arange(128, dtype=np.float32)[None, :], (128, 128)).copy(),
    )
    sched = dict(nA=nA, nB=nB, offA=offA, offB=offB, TA=TA, TB=TB, T=T)
    return sched, shared, per_core, gmap


# ----------------------------------------------------------------- build ----

def build(sched, stage="full", sub=99, repeat=1):
    """stage: one of ph0, adsb, l0, ag, full — truncate program for bisect.
    sub: 1=gathers 3=+St/admm/e 4=+vscale+aggmm 99=full (within emit_layer).
    repeat: emit the whole body N times (device-time delta measurement)."""
    global _SUB
    _SUB = sub
    c = cfg
    H, VW = c.heads, c.val_w
    nA, nB = sched["nA"], sched["nB"]
    offA, offB = sched["offA"], sched["offB"]
    TA, TB, T = sched["TA"], sched["TB"], sched["T"]

    nc = bacc.Bacc("TRN2", target_bir_lowering=False, debug=False,
                   num_devices=c.n_cores,
                   num_swdge_queues=4, dynamic_dma_scratch_size=32768)
    VR = 384                 # val0 row width (bf16 elems)
    E0 = 272                 # val0 e-slot (bf16 col)
    W1R = 128                # val1 row width (bf16 elems)

    def inp(name, shape, dt):
        return nc.dram_tensor(name, list(shape), dt, kind="ExternalInput")

    xT_d = inp("xT", (c.in_ch, c.npad), F32)
    wpack_d = inp("wpack", (c.in_ch, VW + 2 * H), F32)
    b0_d = inp("b0rep", (128, VW), F32)
    g_d = inp("grep", (128, VW), F32)
    bln_d = inp("brep", (128, VW), F32)
    ws_d = inp("wsrep", (128, VW), F32)
    wd_d = inp("wdrep", (128, VW), F32)
    w1c0_d = inp("w1c0rep", (128, VW), F32)
    w1c1_d = inp("w1c1rep", (128, VW), F32)
    b1_d = inp("b1rep", (128, c.out_ch), F32)
    misc_d = inp("misc", (128, 4), F32)
    iota_d = inp("iota", (128, 128), F32)
    iotaP_d = inp("iotaP", (128, 1), BF16)
    idxA_d = inp("idxA", (128, max(TA, 1) * 8), I16)
    idxB_d = inp("idxB", (128, max(TB, 1) * 8), I16)
    idxOlo_d = inp("idxOlo", (128, c.pcpad // 16), I16)
    idxOhi_d = inp("idxOhi", (128, c.pcpad // 16), I16)
    dst_d = inp("dst_cols", (128, max(T, 1)), F32)
    dstTb_d = inp("dstTb", (128, max(T, 1) * 128), BF16)

    val0 = nc.dram_tensor("val0", [c.npad, VR], BF16)
    val1loc = nc.dram_tensor("val1loc", [c.pcpad, W1R], BF16)
    val1 = nc.dram_tensor("val1", [c.npad, W1R], BF16, addr_space="Shared")
    outp = nc.dram_tensor("outp", [c.pcpad, c.out_ch], F32,
                          kind="ExternalOutput")
    dbg = nc.dram_tensor("dbg", [128, 4096], F32)

    groups = list(range(0, c.bpc, c.gb))
    _order = ["ph0", "adsb", "l0", "ag", "full"]
    _lvl = _order.index(stage)

    with tile.TileContext(nc) as tc:
        with (
            tc.tile_pool(name="const", bufs=1) as cpool,
            tc.tile_pool(name="gath", bufs=2) as gpool,
            tc.tile_pool(name="idx", bufs=2) as ipool,
            tc.tile_pool(name="work", bufs=2) as wpool,
            tc.tile_pool(name="flush", bufs=2) as fpool,
        ):
            def cload(ap, shape, dt, tag):
                t = cpool.tile(shape, dt, tag=tag)
                nc.sync.dma_start(t[:], ap[:])
                return t

            wpack_t = cload(wpack_d, [c.in_ch, VW + 2 * H], F32, "wpack")
            b0_t = cload(b0_d, [128, VW], F32, "b0")
            g_t = cload(g_d, [128, VW], F32, "g")
            bln_t = cload(bln_d, [128, VW], F32, "bln")
            ws_t = cload(ws_d, [128, VW], F32, "ws")
            wd_t = cload(wd_d, [128, VW], F32, "wd")
            w1c0_t = cload(w1c0_d, [128, VW], F32, "w1c0")
            w1c1_t = cload(w1c1_d, [128, VW], F32, "w1c1")
            b1_t = cload(b1_d, [128, c.out_ch], F32, "b1")
            misc_t = cload(misc_d, [128, 4], F32, "misc")
            iota_t = cload(iota_d, [128, 128], F32, "iota")
            iotaP_t = cload(iotaP_d, [128, 1], BF16, "iotaP")
            dst_t = cload(dst_d, [128, max(T, 1)], F32, "dstc")
            ad0sb = cpool.tile([128, c.bpc, H], BF16, tag="ad0sb")
            ad1sb = cpool.tile([128, c.bpc, 1], BF16, tag="ad1sb")

            def eps30(hl):
                return misc_t[:, 0:1].broadcast_to([128, hl])

            def onesb(shape):
                v = misc_t[:, 1:2]
                for _ in range(len(shape) - 2):
                    v = v.unsqueeze(2)
                return v.broadcast_to(shape)

            def zerob(shape):
                v = misc_t[:, 2:3]
                for _ in range(len(shape) - 2):
                    v = v.unsqueeze(2)
                return v.broadcast_to(shape)

            def emit_body(rep_i):
                # ---------------- phase 0: node transform for all gids ------
                with (
                    tc.tile_pool(name="psmm", bufs=2, space="PSUM") as psmm,
                    tc.tile_pool(name="xchunk", bufs=2) as xpool,
                    tc.tile_pool(name="ph0", bufs=2) as p0pool,
                ):
                    nblk_tot = c.n_cores * c.bpc
                    for bg in range(0, nblk_tot, 4):
                        nbk = min(4, nblk_tot - bg)
                        xc = xpool.tile([c.in_ch, 4 * 128], F32, tag="xc")
                        nc.sync.dma_start(
                            xc[:, :nbk * 128],
                            xT_d[:, bg * 128:(bg + nbk) * 128])
                        hz = p0pool.tile([128, 4, VR], BF16, tag="hz")
                        for k in range(nbk):
                            ps = psmm.tile([128, VW + 2 * H], F32, tag="ph0ps")
                            nc.tensor.matmul(
                                ps[:], xc[:, k * 128:(k + 1) * 128],
                                wpack_t[:], start=True, stop=True)
                            nc.scalar.activation(hz[:, k, 0:VW], ps[:, 0:VW],
                                                 ACTF.Copy)
                            nc.vector.tensor_tensor(
                                hz[:, k, VW:VW + 4 * H].bitcast(F32),
                                ps[:, VW:VW + 2 * H],
                                zerob([128, 2 * H]), ALU.add)
                        dst_view = val0[bg * 128:(bg + nbk) * 128, 0:E0]\
                            .rearrange("(k p) w -> p k w", p=128)
                        nc.sync.dma_start(dst_view, hz[:, :nbk, 0:E0])

                # ---------------- ad0sb: own ad rows via lo+hi gathers ------
                ntile_own = c.pcpad // 128
                if _lvl >= 1:
                    with tc.tile_pool(name="adbuild", bufs=2) as apool:
                        olo_t = cpool.tile([128, c.pcpad // 16], I16,
                                           tag="olo")
                        nc.sync.dma_start(olo_t[:], idxOlo_d[:])
                        ohi_t = cpool.tile([128, c.pcpad // 16], I16,
                                           tag="ohi")
                        nc.sync.dma_start(ohi_t[:], idxOhi_d[:])
                        CH = 13  # tiles per chunk
                        for t0 in range(0, ntile_own, CH):
                            t1 = min(t0 + CH, ntile_own)
                            nt = t1 - t0
                            glo = apool.tile([128, CH, VR], BF16, tag="glo")
                            nc.gpsimd.dma_gather(
                                out_ap=glo[:, :nt, :],
                                in_ap=val0[0:c.split, :],
                                idxs_ap=olo_t[:, t0 * 8:t1 * 8],
                                num_idxs=nt * 128,
                                num_idxs_reg=nt * 128, elem_size=VR,
                                single_packet=False, queue_num=0)
                            ghi = apool.tile([128, CH, VR], BF16, tag="ghi")
                            nc.gpsimd.dma_gather(
                                out_ap=ghi[:, :nt, :],
                                in_ap=val0[c.split:c.npad, :],
                                idxs_ap=ohi_t[:, t0 * 8:t1 * 8],
                                num_idxs=nt * 128,
                                num_idxs_reg=nt * 128, elem_size=VR,
                                single_packet=False, queue_num=1)
                            gsum = apool.tile([128, CH, 2 * H], F32,
                                              tag="gsum")
                            nc.vector.tensor_tensor(
                                gsum[:, :nt, :],
                                glo[:, :nt, VW:VW + 4 * H].bitcast(F32),
                                ghi[:, :nt, VW:VW + 4 * H].bitcast(F32),
                                ALU.add)
                            nc.vector.tensor_tensor(
                                ad0sb[:, t0:t1, :], gsum[:, :nt, H:2 * H],
                                zerob([128, nt, H]), ALU.add)

                # ---------------- one attention layer -----------------------
                def emit_layer(lyr):
                    HL = H if lyr == 0 else 1
                    valt = val0 if lyr == 0 else val1
                    RW = VR if lyr == 0 else W1R
                    EC = E0 if lyr == 0 else 3
                    adsb = ad0sb if lyr == 0 else ad1sb
                    with (
                        tc.tile_pool(name=f"psagg{lyr}", bufs=2,
                                     space="PSUM") as psagg,
                        tc.tile_pool(name=f"psad{lyr}", bufs=2,
                                     space="PSUM") as psad,
                    ):
                        for g0 in groups:
                            emit_group(lyr, HL, valt, RW, EC, adsb,
                                       psagg, psad, g0)

                def emit_group(lyr, HL, valt, RW, EC, adsb, psagg, psad, g0):
                    g1 = min(g0 + c.gb, c.bpc)
                    blks = range(g0, g1)
                    tA0, tA1 = int(offA[g0]), int(offA[g1])
                    tB0, tB1 = int(offB[g0]), int(offB[g1])
                    nAg, nBg = tA1 - tA0, tB1 - tB0
                    nG = nAg + nBg
                    if nG == 0:
                        return
                    tD0 = tA0 + tB0
                    # idx slices + replicated dst row slice
                    if nAg:
                        iA = ipool.tile([128, nAg * 8], I16, tag="iA")
                        nc.sync.dma_start(iA[:], idxA_d[:, tA0 * 8:tA1 * 8])
                    if nBg:
                        iB = ipool.tile([128, nBg * 8], I16, tag="iB")
                        nc.sync.dma_start(iB[:], idxB_d[:, tB0 * 8:tB1 * 8])
                    dT = ipool.tile([128, nG, 128], BF16, tag="dT")
                    nc.sync.dma_start(
                        dT[:], dstTb_d[:, tD0 * 128:(tD0 + nG) * 128]
                        .rearrange("p (t j) -> p t j", j=128))

                    # value gathers from A/B halves
                    vA = gpool.tile([128, max(nAg, 1), RW], BF16, tag="vA")
                    if nAg:
                        nc.gpsimd.dma_gather(
                            out_ap=vA[:], in_ap=valt[0:c.split, :],
                            idxs_ap=iA[:], num_idxs=nAg * 128,
                            num_idxs_reg=nAg * 128, elem_size=RW,
                            single_packet=False, queue_num=0)
                    vB = gpool.tile([128, max(nBg, 1), RW], BF16, tag="vB")
                    if nBg:
                        nc.gpsimd.dma_gather(
                            out_ap=vB[:], in_ap=valt[c.split:c.npad, :],
                            idxs_ap=iB[:], num_idxs=nBg * 128,
                            num_idxs_reg=nBg * 128, elem_size=RW,
                            single_packet=False, queue_num=1)

                    if _SUB == 1:   # consume gathers, skip compute
                        nc.sync.dma_start(dbg[:, 0:64], dT[:, 0, 0:128]
                                          .bitcast(F32))
                        if nAg:
                            nc.sync.dma_start(
                                dbg[:, 512:512 + RW // 2],
                                vA[:, 0, :].bitcast(F32))
                        return

                    # St[j, t, slot] = (dst[t, slot] == j), lhsT for ad-mm
                    st_t = wpool.tile([128, nG, 128], BF16, tag="St")
                    nc.vector.tensor_tensor(
                        st_t[:], dT[:],
                        iotaP_t[:].unsqueeze(2).broadcast_to([128, nG, 128]),
                        ALU.is_equal)
                    # ad_slot = St^T @ adblock per tile
                    adps = psad.tile([128, nG, HL], F32, tag=f"adps{lyr}")
                    pos = 0
                    for b in blks:
                        tb = int(nA[b]) + int(nB[b])
                        for t in range(tb):
                            nc.tensor.matmul(
                                adps[:, pos + t, :], st_t[:, pos + t, :],
                                adsb[:, b, 0:HL], start=True, stop=True)
                        pos += tb

                    # e = exp(lrelu(as + ad)) -> row e-slots, per block seg
                    logit = wpool.tile([128, nG, HL], F32, tag="logit")
                    pos = 0
                    for b in blks:
                        na, nb_ = int(nA[b]), int(nB[b])
                        for vv, nseg, toff in (
                                (vA, na, int(offA[b]) - tA0),
                                (vB, nb_, int(offB[b]) - tB0)):
                            if nseg == 0:
                                continue
                            if lyr == 0:
                                asv = vv[:, toff:toff + nseg, VW:VW + 2 * H]\
                                    .bitcast(F32)[:, :, 0:HL]
                            else:
                                asv = vv[:, toff:toff + nseg, 4:6]\
                                    .bitcast(F32)[:, :, 0:HL]
                            nc.vector.tensor_tensor(
                                logit[:, pos:pos + nseg, :], asv,
                                adps[:, pos:pos + nseg, :], ALU.add)
                            pos += nseg
                    lr = wpool.tile([128, nG, HL], F32, tag="lr")
                    nc.vector.scalar_tensor_tensor(
                        lr[:], logit[:], float(c.neg), logit[:],
                        ALU.mult, ALU.max)
                    pos = 0
                    for b in blks:
                        na, nb_ = int(nA[b]), int(nB[b])
                        for vv, nseg, toff in (
                                (vA, na, int(offA[b]) - tA0),
                                (vB, nb_, int(offB[b]) - tB0)):
                            if nseg == 0:
                                continue
                            nc.scalar.activation(
                                vv[:, toff:toff + nseg, EC:EC + HL],
                                lr[:, pos:pos + nseg, :], ACTF.Exp)
                            pos += nseg

                    if _SUB == 3:   # consume e, skip the rest
                        if nAg:
                            ce = wpool.tile([128, nAg], F32, tag="ce")
                            nc.vector.tensor_tensor(
                                ce[:], vA[:, :, EC],
                                zerob([128, nAg]), ALU.add)
                            nc.sync.dma_start(dbg[:, 0:nAg], ce[:])
                        return

                    # V' = V * e (per head), in place
                    nval = VW if lyr == 0 else 2
                    for vv, nseg in ((vA, nAg), (vB, nBg)):
                        if nseg == 0:
                            continue
                        vview = vv[:, :, 0:nval].rearrange(
                            "p t (h c) -> p t h c", h=HL)
                        ee = vv[:, :, EC:EC + HL].unsqueeze(3)
                        nc.vector.tensor_tensor(
                            vview, vview,
                            ee.broadcast_to([128, nseg, HL, nval // HL]),
                            ALU.mult)

                    # per block: S build, agg matmul (den folded), flush
                    NAGG = EC + HL  # rhs cols: values | (junk) | e
                    pos = 0
                    for b in blks:
                        na, nb_ = int(nA[b]), int(nB[b])
                        tb = na + nb_
                        if tb == 0:
                            continue
                        tcol0 = tD0 + pos
                        s_t = wpool.tile([128, tb * 128], BF16, tag="S")
                        nc.vector.tensor_tensor(
                            s_t[:].rearrange("p (t j) -> p t j", j=128),
                            dst_t[:, tcol0:tcol0 + tb].unsqueeze(2)
                            .broadcast_to([128, tb, 128]),
                            iota_t[:].unsqueeze(1)
                            .broadcast_to([128, tb, 128]),
                            ALU.is_equal)
                        agg = psagg.tile([128, NAGG], F32, tag=f"agg{lyr}")
                        for t in range(tb):
                            lhs = s_t[:, t * 128:(t + 1) * 128]
                            if t < na:
                                vv = vA[:, int(offA[b]) - tA0 + t, 0:NAGG]
                            else:
                                vv = vB[:, int(offB[b]) - tB0 + (t - na),
                                        0:NAGG]
                            nc.tensor.matmul(agg[:], lhs, vv,
                                             start=(t == 0),
                                             stop=(t == tb - 1))
                        pos += tb

                        if _SUB == 4:   # consume agg, skip flush
                            ca = fpool.tile([128, NAGG], F32, tag="ca")
                            nc.scalar.activation(ca[:], agg[:], ACTF.Copy)
                            nc.sync.dma_start(dbg[:, 0:NAGG], ca[:])
                            continue
                        if lyr == 0:
                            flush0(agg, b)
                        else:
                            flush1(agg, b)

                # ---- layer-0 flush: +b0, LayerNorm, ELU, project, tables ---
                def flush0(agg, b):
                    HL = H
                    rows = slice(b * 128, (b + 1) * 128)
                    den = agg[:, E0:E0 + HL]
                    deneps = fpool.tile([128, HL], F32, tag="deneps")
                    nc.vector.tensor_tensor(deneps[:], den, eps30(HL), ALU.add)
                    rcp = fpool.tile([128, HL], F32, tag="rcp")
                    nc.vector.reciprocal(rcp[:], deneps[:])
                    sc = fpool.tile([128, VW], F32, tag="sc")
                    nc.vector.tensor_tensor(
                        sc[:].rearrange("p (h c) -> p h c", h=HL),
                        agg[:, 0:VW].rearrange("p (h c) -> p h c", h=HL),
                        rcp[:].unsqueeze(2).broadcast_to(
                            [128, HL, VW // HL]),
                        ALU.mult)
                    nc.vector.tensor_tensor(sc[:], sc[:], b0_t[:], ALU.add)
                    if _SUB == 5:
                        nc.sync.dma_start(dbg[:, 0:VW], sc[:])
                        return
                    # LayerNorm on ACT: Identity-with-bias / Square-accum /
                    # Sqrt-with-scale-bias; reciprocal on DVE
                    mu = fpool.tile([128, 1], F32, tag="mu")
                    nc.vector.tensor_reduce(
                        mu[:], sc[:], mybir.AxisListType.X, ALU.add)
                    negmu = fpool.tile([128, 1], F32, tag="negmu")
                    nc.vector.scalar_tensor_tensor(
                        negmu[:], mu[:], -1.0 / VW, misc_t[:, 2:3],
                        ALU.mult, ALU.add)
                    xc_ = fpool.tile([128, VW], F32, tag="xc0")
                    nc.scalar.activation(xc_[:], sc[:], ACTF.Identity,
                                         bias=negmu[:])
                    if _SUB == 6:
                        nc.sync.dma_start(dbg[:, 0:VW], xc_[:])
                        return
                    sq = fpool.tile([128, VW], F32, tag="sq")
                    var = fpool.tile([128, 1], F32, tag="var")
                    nc.scalar.activation(sq[:], xc_[:], ACTF.Square,
                                         accum_out=var[:])
                    if _SUB == 61:
                        nc.sync.dma_start(dbg[:, 0:1], var[:])
                        return
                    sd = fpool.tile([128, 1], F32, tag="sd")
                    nc.scalar.activation(sd[:], var[:], ACTF.Sqrt,
                                         bias=misc_t[:, 3:4], scale=1.0 / VW)
                    rstd = fpool.tile([128, 1], F32, tag="rstd")
                    nc.vector.reciprocal(rstd[:], sd[:])
                    if _SUB == 62:
                        nc.sync.dma_start(dbg[:, 0:1], rstd[:])
                        return
                    zz = fpool.tile([128, VW], F32, tag="zz")
                    nc.vector.scalar_tensor_tensor(
                        zz[:], xc_[:], rstd[:], g_t[:], ALU.mult, ALU.mult)
                    nc.vector.tensor_tensor(zz[:], zz[:], bln_t[:], ALU.add)
                    # ELU: z = relu(x) + exp(x - relu(x)) - 1
                    zmax = fpool.tile([128, VW], F32, tag="zmax")
                    nc.scalar.activation(zmax[:], zz[:], ACTF.Relu)
                    zmin = fpool.tile([128, VW], F32, tag="zmin")
                    nc.vector.tensor_tensor(zmin[:], zz[:], zmax[:],
                                            ALU.subtract)
                    pexp = fpool.tile([128, VW], F32, tag="pexp")
                    nc.scalar.activation(pexp[:], zmin[:], ACTF.Exp)
                    z = fpool.tile([128, VW], F32, tag="z")
                    nc.vector.tensor_tensor(z[:], zmax[:], pexp[:], ALU.add)
                    nc.vector.tensor_tensor(z[:], z[:], onesb([128, VW]),
                                            ALU.subtract)
                    if _SUB == 7:   # stop after ELU
                        nc.sync.dma_start(dbg[:, 0:VW], z[:])
                        return
                    # project 256->2 + attention scalars, as 4 fused dots
                    scr = fpool.tile([128, VW], F32, tag="scr")
                    stage = fpool.tile([128, W1R], BF16, tag="v1stage")
                    dd = fpool.tile([128, 4], F32, tag="dd")
                    for j, wt in enumerate((w1c0_t, w1c1_t, ws_t, wd_t)):
                        nc.vector.tensor_tensor(scr[:], z[:], wt[:], ALU.mult)
                        nc.vector.tensor_reduce(
                            dd[:, j:j + 1], scr[:], mybir.AxisListType.X,
                            ALU.add)
                    if _SUB == 8:
                        nc.sync.dma_start(dbg[:, 0:4], dd[:])
                        return
                    nc.vector.tensor_tensor(
                        stage[:, 0:2], dd[:, 0:2], zerob([128, 2]), ALU.add)
                    nc.vector.tensor_tensor(
                        stage[:, 2:4], zerob([128, 2]), zerob([128, 2]),
                        ALU.add)
                    nc.vector.tensor_tensor(
                        stage[:, 4:6].bitcast(F32), dd[:, 2:3],
                        zerob([128, 1]), ALU.add)
                    nc.vector.tensor_tensor(
                        stage[:, 6:W1R], zerob([128, W1R - 6]),
                        zerob([128, W1R - 6]), ALU.add)
                    nc.vector.tensor_tensor(
                        ad1sb[:, b, :], dd[:, 3:4], zerob([128, 1]), ALU.add)
                    dstv = val1loc[rows, :].rearrange("(k p) w -> p k w",
                                                      p=128)
                    nc.sync.dma_start(dstv, stage[:].unsqueeze(1))

                # ---- layer-1 flush: out = agg/den + b1 ---------------------
                def flush1(agg, b):
                    rows = slice(b * 128, (b + 1) * 128)
                    deneps = fpool.tile([128, 1], F32, tag="deneps1")
                    nc.vector.tensor_tensor(deneps[:], agg[:, 3:4], eps30(1),
                                            ALU.add)
                    rcp = fpool.tile([128, 1], F32, tag="rcp1")
                    nc.vector.reciprocal(rcp[:], deneps[:])
                    ob = fpool.tile([128, c.out_ch], F32, tag="ob")
                    nc.vector.scalar_tensor_tensor(
                        ob[:], agg[:, 0:c.out_ch], rcp[:], b1_t[:],
                        ALU.mult, ALU.add)
                    nc.sync.dma_start(outp[rows, :], ob[:])

                if _lvl >= 2:
                    emit_layer(0)
                if _lvl >= 3:
                    nc.gpsimd.collective_compute(
                        "AllGather", ALU.bypass,
                        replica_groups=[list(range(c.n_cores))],
                        ins=[val1loc[:]], outs=[val1[:]])
                if _lvl >= 4:
                    emit_layer(1)

            for _rep in range(repeat):
                emit_body(_rep)

    nc.compile()
    return nc


# ------------------------------------------------------------------ run -----

_cache = {}
_SUB = 99


def kernel(**inputs):
    c = cfg
    sched, shared, per_core, gmap = prep(**inputs)
    key = _sched_sig(sched)
    if key in _cache:
        nc = _cache[key]
    else:
        nc = build(sched)
        _cache[key] = nc

    in_maps = []
    for ci in range(c.n_cores):
        m = dict(shared)
        m.update(per_core[ci])
        in_maps.append(m)
    res = run_bass_kernel_spmd(nc, in_maps, list(range(c.n_cores)))
    out = np.zeros((c.n_nodes, c.out_ch), np.float32)
    loc = gmap % c.pcpad
    for ci in range(c.n_cores):
        sel = slice(ci * c.pc, (ci + 1) * c.pc)
        out[sel] = res.results[ci]["outp"][loc[sel]]
    return out.astype(np.float32)


def _sched_sig(s):
    return (tuple(s["nA"].tolist()), tuple(s["nB"].tolist()))
